# revision 23
# baseline (speedup 1.0000x reference)
"""DEMA (double exponential moving average) Trainium2 Bass kernel.

Problem: x [32, 4096, 512] f32; y = 2*EMA(x) - EMA(EMA(x)) along time axis
(L=4096), alpha=0.1, with y_0 = x_0 initial condition.

Strategy
--------
Data-parallel over batch: 8 cores x 4 batch rows each (no communication).

DEMA is a linear recurrence with a 2-dim state (the two EMA carries c1, c2).
Per core the time axis is processed in blocks of T=126 steps: one constant
augmented matrix A [128, 128] maps [c1; c2; x_block(126)] ->
[c1'; c2'; dema_block(126)], so each block is exactly ONE matmul on the
tensor engine. Blocks chain via the 2 carry rows.

The problem is memory-bound (tolerance 2e-2), so all HBM traffic is bf16:
the host converts x f32->bf16 pre-blocked into contiguous slabs, the device
computes bf16 matmuls with fp32 PSUM accumulation and stores bf16, and the
host converts back to f32. This halves DMA bytes vs f32 (16.8 MB in +
16.8 MB out per core).

Layout: the 4 batch rows are processed as 2 BATCH PAIRS. Each SBUF tile
holds one pair's group of 11 time-blocks [128, 2*11*512] bf16, so
- every load/store is one fully-contiguous 2.77 MB DMA (6 loads + 6 stores
  per core), amortizing the ~2us per-DMA fixed cost;
- the two blocks (one per batch) at the same position share one 2-bank PSUM
  tile [128, 1024], so PSUM->SBUF traffic needs only ONE output copy (ACT)
  and ONE carry copy (DVE) per position: 66 + 66 copies instead of
  132 + 129, halving per-instruction overhead on both engines;
- the two pairs form two independent carry chains that interleave to keep
  every engine busy.

Partition layout: carries at partitions 0..1 (compute APs may start at 0),
time row t at partition t+2 in order -- loads/stores need no partition
splits.
"""

import numpy as np

ALPHA = 0.1
BETA = 1.0 - ALPHA
B_FULL, L, C = 32, 4096, 512
N_CORES = 8
B_PER_CORE = B_FULL // N_CORES  # 4
NPAIR = B_PER_CORE // 2  # 2 batch pairs per core
T = 126  # time steps per block (plus 2 carry rows = 128 partitions)
NBLK = 33  # 32 full blocks + 1 zero-padded tail block (64 valid rows)
LPAD = NBLK * T  # 4158
GRP = 11  # blocks per group (one SBUF tile / one DMA per (pair, group))
NG = NBLK // GRP  # 3 groups
W = GRP * C  # 5632 free elements per batch per group
PW = 2 * W  # 11264 free elements per pair tile


def _build_A_raw(dtype=np.float64):
    """Raw augmented operator; index layout [x rows 0..T-1, c1 at T, c2 at
    T+1]."""
    i = np.arange(T)
    M = np.zeros((T, T), dtype)
    for r in range(T):
        M[r, : r + 1] = ALPHA * BETA ** (r - np.arange(r + 1))
    d = BETA ** (i + 1.0)
    M2 = M @ M
    Md = M @ d
    A = np.zeros((T + 2, T + 2), dtype)
    A[:T, :T] = 2 * M - M2
    A[:T, T] = 2 * d - Md
    A[:T, T + 1] = -d
    A[T, :T] = M[T - 1, :]
    A[T, T] = BETA**T
    A[T + 1, :T] = M2[T - 1, :]
    A[T + 1, T] = Md[T - 1]
    A[T + 1, T + 1] = BETA**T
    return A


# permutation: partition 0 <- c1, partition 1 <- c2, partition t+2 <- time t
_ORDER = [T, T + 1] + list(range(T))


def _build_mats():
    """Returns (A_perm, A0_perm) f64. A0 folds the c1 = c2 = x_0 initial
    condition into the x_0 column so block 0 needs no carry input (the carry
    partitions only need to hold finite values)."""
    A = _build_A_raw()
    A0 = A.copy()
    A0[:, 0] += A[:, T] + A[:, T + 1]
    A0[:, T] = 0.0
    A0[:, T + 1] = 0.0
    ix = np.ix_(_ORDER, _ORDER)
    return A[ix], A0[ix]


def _to_bf16_u16(a):
    """f32 ndarray (contiguous) -> uint16 bf16 bits, round-to-nearest-even."""
    a = np.ascontiguousarray(a, dtype=np.float32)
    v = a.view(np.uint32)
    r = ((v >> np.uint32(16)) & np.uint32(1)) + np.uint32(0x7FFF)
    return ((v + r) >> np.uint32(16)).astype(np.uint16)


def _bf16_dtype():
    import concourse.mybir as mybir

    return mybir.dt.np(mybir.dt.bfloat16)


def _amat_np():
    """Both lhsT matrices packed as one [128, 256] bf16 input."""
    Ap, A0p = _build_mats()
    out = np.zeros((128, 256), dtype=np.uint16)
    out[:, 0:128] = _to_bf16_u16(np.ascontiguousarray(Ap.T))
    out[:, 128:256] = _to_bf16_u16(np.ascontiguousarray(A0p.T))
    return out.view(_bf16_dtype())


def _repack_x(x):
    """x [B, L, C] f32 -> device layout [B//2 pairs, NG, T, PW] bf16.

    Block n holds time rows n*T..n*T+125 (block 32 zero-padded past row 63).
    Pair p = batches (2p, 2p+1); free axis is (batch-in-pair, block, chan),
    so each (p, g) slab [T, PW] is exactly the SBUF tile image (partition
    t+2 <- row t), fully contiguous in DRAM."""
    b = x.shape[0]
    xu = _to_bf16_u16(x).reshape(b, L, C)
    xb = np.zeros((b, NBLK, T, C), np.uint16)
    nfull = L // T  # 32
    xb[:, :nfull] = xu[:, : nfull * T].reshape(b, nfull, T, C)
    xb[:, nfull, : L - nfull * T] = xu[:, nfull * T :]
    xdev = np.ascontiguousarray(
        xb.reshape(b // 2, 2, NG, GRP, T, C).transpose(0, 2, 4, 1, 3, 5)
    ).reshape(b // 2, NG, T, PW)
    return xdev.view(_bf16_dtype())


def _unpack_y(y_dev_u16):
    """Device layout [B//2, NG, T, PW] bf16-as-u16 -> y [B, L, C] f32."""
    p = y_dev_u16.shape[0]
    yb = (
        y_dev_u16.reshape(p, NG, T, 2, GRP, C)
        .transpose(0, 3, 1, 4, 2, 5)
        .reshape(2 * p, LPAD, C)[:, :L]
    )
    yb = np.ascontiguousarray(yb)
    return (yb.astype(np.uint32) << np.uint32(16)).view(np.float32)


def build_bass(loop_iters=1, load_split=(1, 4), store_split=(4, 8), copy_scheme="carry_from_ot"):
    """Emit the per-core Bass/Tile program. Returns the Bass module.

    loop_iters > 1 wraps the whole kernel body in a hardware For_i loop that
    re-executes it loop_iters times on the same data (identical per-iteration
    instruction stream + ~2us back-edge). Only used by test.py to get a
    large, dispatch-noise-immune timing signal."""
    import concourse.bass as bass
    import concourse.bacc as bacc
    import concourse.mybir as mybir
    from concourse import tile
    from contextlib import nullcontext

    ng = NG
    fp32 = mybir.dt.float32
    bf16 = mybir.dt.bfloat16
    nc = bacc.Bacc(
        "TRN2", target_bir_lowering=False, debug=False, num_devices=N_CORES
    )

    x = nc.dram_tensor("x", [NPAIR, ng, T, PW], bf16, kind="ExternalInput")
    # amat[:, 0:128] = steady-state lhsT; amat[:, 128:256] = first-block lhsT.
    # The loop variant pads amat by loop_iters columns: the jax-side NEFF
    # cache fingerprints only the HLO (shapes), not the embedded BIR, so the
    # timing variant must differ in shape to avoid a false NEFF-cache hit.
    amat_cols = 256 + (loop_iters if loop_iters > 1 else 0)
    amat = nc.dram_tensor("amat", [128, amat_cols], bf16, kind="ExternalInput")
    y = nc.dram_tensor("y", [NPAIR, ng, T, PW], bf16, kind="ExternalOutput")
    x_ap, y_ap = x.ap(), y.ap()

    def pair3(ap_2d):
        """[P, PW] AP -> [P, 2, W] view (batch-in-pair axis split out)."""
        return ap_2d.rearrange("t (b f) -> t b f", b=2)

    with tile.TileContext(nc) as tc:
        with (
            tc.tile_pool(name="w", bufs=1) as w_pool,
            tc.tile_pool(name="rhs", bufs=2 * NPAIR) as rhs_pool,
            tc.tile_pool(name="out", bufs=2 * NPAIR) as out_pool,
            tc.tile_pool(name="psum", bufs=4, space="PSUM") as psum_pool,
        ):
            w = w_pool.tile([128, 256], bf16)
            nc.sync.dma_start(w[:, :], amat.ap()[:, 0:256])

            loop_cm = (
                tc.For_i(0, loop_iters, 1) if loop_iters > 1 else nullcontext()
            )

            def load_group(p, g):
                """rhs tile for (pair p, group g): contiguous DMA(s) into
                partitions 2..127; carries (partitions 0..1) are written by
                the previous block's carry copy. Group 0 is split into small
                head chunks so the first matmuls start after ~2us instead of
                waiting for the full 2.77 MB transfer."""
                rt = rhs_pool.tile([128, PW], bf16, name="rt")
                bounds = (0,) + tuple(load_split) + (GRP,) if g == 0 else (0, GRP)
                xg3 = x_ap[p, g, :, :].rearrange("t (b f) -> t b f", b=2)
                for lo, hi in zip(bounds[:-1], bounds[1:]):
                    cols = slice(lo * C, hi * C)
                    nc.sync.dma_start(
                        pair3(rt[2:128, :])[:, :, cols], xg3[:, :, cols]
                    )
                if g == 0:
                    # block 0 uses the A0 matrix (zero carry columns); its
                    # carry partitions just need to be finite
                    nc.gpsimd.memset(pair3(rt[0:2, :])[:, :, 0:C], 0.0)
                return rt

            with loop_cm:
                rhs_cur = [load_group(p, 0) for p in range(NPAIR)]

                for g in range(ng):
                    rhs_nxt = [
                        load_group(p, g + 1) if g + 1 < ng else None
                        for p in range(NPAIR)
                    ]
                    ots = [
                        out_pool.tile([128, PW], bf16, name="ot")
                        for p in range(NPAIR)
                    ]
                    for k in range(GRP):
                        for p in range(NPAIR):
                            rt = rhs_cur[p]
                            ps = psum_pool.tile([128, 2 * C], fp32)
                            first_block = g == 0 and k == 0
                            lhsT = w[:, 128:256] if first_block else w[:, 0:128]
                            # the pair's two blocks at position k -> the two
                            # halves of one 2-bank PSUM tile
                            nc.tensor.matmul(
                                ps[:, 0:C], lhsT, rt[:, k * C : (k + 1) * C],
                                start=True, stop=True,
                            )
                            nc.tensor.matmul(
                                ps[:, C : 2 * C], lhsT,
                                rt[:, W + k * C : W + (k + 1) * C],
                                start=True, stop=True,
                            )
                            if copy_scheme in (
                                "pair_act_dve", "split_carry", "carry_from_ot"
                            ):
                                out_eng = nc.scalar.copy
                                carry_eng = nc.vector.tensor_copy
                            elif copy_scheme == "chain_per_engine":
                                # chain p0 entirely on ACT, chain p1 on DVE:
                                # no head-of-line coupling between chains
                                out_eng = (
                                    nc.scalar.copy if p == 0
                                    else nc.vector.tensor_copy
                                )
                                carry_eng = out_eng
                            elif copy_scheme == "crossed":
                                out_eng = (
                                    nc.scalar.copy if p == 0
                                    else nc.vector.tensor_copy
                                )
                                carry_eng = (
                                    nc.vector.tensor_copy if p == 0
                                    else nc.scalar.copy
                                )
                            else:
                                raise ValueError(copy_scheme)
                            # ONE cast copy for both blocks (incl. carry rows,
                            # harmless)
                            out_eng(
                                pair3(ots[p][:, :])[:, :, k * C : (k + 1) * C],
                                pair3(ps[:, :]),
                            )
                            # carry copy/copies into the next block's rhs
                            if k + 1 < GRP:
                                ct, cols = rhs_cur[p], slice(
                                    (k + 1) * C, (k + 2) * C
                                )
                            elif rhs_nxt[p] is not None:
                                ct, cols = rhs_nxt[p], slice(0, C)
                            else:
                                ct = None
                            if ct is not None and copy_scheme == "carry_from_ot":
                                # carry from the already-cast bf16 out tile,
                                # bitcast to u32: DVE 2x_2P mode makes this
                                # ~3x cheaper than the fp32 PSUM read; bits
                                # identical to casting from PSUM directly
                                import concourse.mybir as mybir

                                src = pair3(ots[p][0:2, :])[
                                    :, :, k * C : (k + 1) * C
                                ].bitcast(mybir.dt.uint32)
                                carry_eng(
                                    pair3(ct[0:2, :])[:, :, cols].bitcast(
                                        mybir.dt.uint32
                                    ),
                                    src,
                                )
                            elif ct is not None and copy_scheme == "split_carry":
                                # per-batch carries: 4 independent short
                                # chains instead of 2 coupled ones
                                for b in range(2):
                                    carry_eng(
                                        ct[
                                            0:2,
                                            b * W + cols.start : b * W + cols.stop,
                                        ],
                                        ps[0:2, b * C : (b + 1) * C],
                                    )
                            elif ct is not None:
                                carry_eng(
                                    pair3(ct[0:2, :])[:, :, cols],
                                    pair3(ps[0:2, :]),
                                )
                        # store finished chunks early (SWDGE/Pool ring keeps
                        # store DMA time off the ACT/SP queues); splitting
                        # drains the last group incrementally
                        if k + 1 in store_split or k + 1 == GRP:
                            prev = max(
                                [0] + [s for s in store_split if s < k + 1]
                            )
                            cols = slice(prev * C, (k + 1) * C)
                            for p in range(NPAIR):
                                yg3 = y_ap[p, g, :, :].rearrange(
                                    "t (b f) -> t b f", b=2
                                )
                                nc.gpsimd.dma_start(
                                    yg3[:, :, cols],
                                    pair3(ots[p][2:128, :])[:, :, cols],
                                )
                    rhs_cur = rhs_nxt
    nc.compile()
    return nc


_CACHED = {}


def _get_nc():
    if "nc" not in _CACHED:
        _CACHED["nc"] = build_bass()
    return _CACHED["nc"]


def _core_in_maps(xdev, amat):
    return [
        {"x": xdev[i * NPAIR : (i + 1) * NPAIR], "amat": amat}
        for i in range(N_CORES)
    ]


def kernel(**inputs: np.ndarray) -> np.ndarray:
    from concourse.bass_utils import run_bass_kernel_spmd

    x = np.ascontiguousarray(inputs["x"], dtype=np.float32)
    assert x.shape == (B_FULL, L, C), x.shape

    xdev = _repack_x(x)
    amat = _amat_np()

    nc = _get_nc()
    res = run_bass_kernel_spmd(
        nc, _core_in_maps(xdev, amat), core_ids=list(range(N_CORES))
    )
    y_u16 = np.concatenate(
        [np.ascontiguousarray(r["y"]).view(np.uint16) for r in res.results],
        axis=0,
    )
    return _unpack_y(y_u16)


# revision 24
# speedup vs baseline: 1.0436x; 1.0436x over previous
"""DEMA (double exponential moving average) Trainium2 Bass kernel.

Problem: x [32, 4096, 512] f32; y = 2*EMA(x) - EMA(EMA(x)) along time axis
(L=4096), alpha=0.1, with y_0 = x_0 initial condition.

Strategy
--------
Data-parallel over batch: 8 cores x 4 batch rows each (no communication).

DEMA is a linear recurrence with a 2-dim state (the two EMA carries c1, c2).
Per core the time axis is processed in blocks of T=126 steps: one constant
augmented matrix A [128, 128] maps [c1; c2; x_block(126)] ->
[c1'; c2'; dema_block(126)], so each block is exactly ONE matmul on the
tensor engine. Blocks chain via the 2 carry rows.

The problem is memory-bound (tolerance 2e-2), so all HBM traffic is bf16:
the host converts x f32->bf16 pre-blocked into contiguous slabs, the device
computes bf16 matmuls with fp32 PSUM accumulation and stores bf16, and the
host converts back to f32. This halves DMA bytes vs f32 (16.8 MB in +
16.8 MB out per core).

Layout: the 4 batch rows are processed as 2 BATCH PAIRS. Each SBUF tile
holds one pair's group of 11 time-blocks [128, 2*11*512] bf16, so
- every load/store is one fully-contiguous 2.77 MB DMA (6 loads + 6 stores
  per core), amortizing the ~2us per-DMA fixed cost;
- the two blocks (one per batch) at the same position share one 2-bank PSUM
  tile [128, 1024], so PSUM->SBUF traffic needs only ONE output copy (ACT)
  and ONE carry copy (DVE) per position: 66 + 66 copies instead of
  132 + 129, halving per-instruction overhead on both engines;
- the two pairs form two independent carry chains that interleave to keep
  every engine busy.

Partition layout: carries at partitions 0..1 (compute APs may start at 0),
time row t at partition t+2 in order -- loads/stores need no partition
splits.
"""

import numpy as np

ALPHA = 0.1
BETA = 1.0 - ALPHA
B_FULL, L, C = 32, 4096, 512
N_CORES = 8
B_PER_CORE = B_FULL // N_CORES  # 4
NPAIR = B_PER_CORE // 2  # 2 batch pairs per core
T = 126  # time steps per block (plus 2 carry rows = 128 partitions)
NBLK = 33  # 32 full blocks + 1 zero-padded tail block (64 valid rows)
LPAD = NBLK * T  # 4158
GRP = 11  # blocks per group (one SBUF tile / one DMA per (pair, group))
NG = NBLK // GRP  # 3 groups
W = GRP * C  # 5632 free elements per batch per group
PW = 2 * W  # 11264 free elements per pair tile


def _build_A_raw(dtype=np.float64):
    """Raw augmented operator; index layout [x rows 0..T-1, c1 at T, c2 at
    T+1]."""
    i = np.arange(T)
    M = np.zeros((T, T), dtype)
    for r in range(T):
        M[r, : r + 1] = ALPHA * BETA ** (r - np.arange(r + 1))
    d = BETA ** (i + 1.0)
    M2 = M @ M
    Md = M @ d
    A = np.zeros((T + 2, T + 2), dtype)
    A[:T, :T] = 2 * M - M2
    A[:T, T] = 2 * d - Md
    A[:T, T + 1] = -d
    A[T, :T] = M[T - 1, :]
    A[T, T] = BETA**T
    A[T + 1, :T] = M2[T - 1, :]
    A[T + 1, T] = Md[T - 1]
    A[T + 1, T + 1] = BETA**T
    return A


# permutation: partition 0 <- c1, partition 1 <- c2, partition t+2 <- time t
_ORDER = [T, T + 1] + list(range(T))


def _build_mats():
    """Returns (A_perm, A0_perm) f64. A0 folds the c1 = c2 = x_0 initial
    condition into the x_0 column so block 0 needs no carry input (the carry
    partitions only need to hold finite values)."""
    A = _build_A_raw()
    A0 = A.copy()
    A0[:, 0] += A[:, T] + A[:, T + 1]
    A0[:, T] = 0.0
    A0[:, T + 1] = 0.0
    ix = np.ix_(_ORDER, _ORDER)
    return A[ix], A0[ix]


def _to_bf16_u16(a):
    """f32 ndarray (contiguous) -> uint16 bf16 bits, round-to-nearest-even."""
    a = np.ascontiguousarray(a, dtype=np.float32)
    v = a.view(np.uint32)
    r = ((v >> np.uint32(16)) & np.uint32(1)) + np.uint32(0x7FFF)
    return ((v + r) >> np.uint32(16)).astype(np.uint16)


def _bf16_dtype():
    import concourse.mybir as mybir

    return mybir.dt.np(mybir.dt.bfloat16)


def _amat_np():
    """Both lhsT matrices packed as one [128, 256] bf16 input."""
    Ap, A0p = _build_mats()
    out = np.zeros((128, 256), dtype=np.uint16)
    out[:, 0:128] = _to_bf16_u16(np.ascontiguousarray(Ap.T))
    out[:, 128:256] = _to_bf16_u16(np.ascontiguousarray(A0p.T))
    return out.view(_bf16_dtype())


def _repack_x(x):
    """x [B, L, C] f32 -> device layout [B//2 pairs, NG, T, PW] bf16.

    Block n holds time rows n*T..n*T+125 (block 32 zero-padded past row 63).
    Pair p = batches (2p, 2p+1); free axis is (batch-in-pair, block, chan),
    so each (p, g) slab [T, PW] is exactly the SBUF tile image (partition
    t+2 <- row t), fully contiguous in DRAM."""
    b = x.shape[0]
    xu = _to_bf16_u16(x).reshape(b, L, C)
    xb = np.zeros((b, NBLK, T, C), np.uint16)
    nfull = L // T  # 32
    xb[:, :nfull] = xu[:, : nfull * T].reshape(b, nfull, T, C)
    xb[:, nfull, : L - nfull * T] = xu[:, nfull * T :]
    xdev = np.ascontiguousarray(
        xb.reshape(b // 2, 2, NG, GRP, T, C).transpose(0, 2, 4, 1, 3, 5)
    ).reshape(b // 2, NG, T, PW)
    return xdev.view(_bf16_dtype())


def _unpack_y(y_dev_u16):
    """Device layout [B//2, NG, T, PW] bf16-as-u16 -> y [B, L, C] f32."""
    p = y_dev_u16.shape[0]
    yb = (
        y_dev_u16.reshape(p, NG, T, 2, GRP, C)
        .transpose(0, 3, 1, 4, 2, 5)
        .reshape(2 * p, LPAD, C)[:, :L]
    )
    yb = np.ascontiguousarray(yb)
    return (yb.astype(np.uint32) << np.uint32(16)).view(np.float32)


def build_bass(loop_iters=1, load_split=(1, 4), store_split=(4, 8), copy_scheme="split_carry"):
    """Emit the per-core Bass/Tile program. Returns the Bass module.

    loop_iters > 1 wraps the whole kernel body in a hardware For_i loop that
    re-executes it loop_iters times on the same data (identical per-iteration
    instruction stream + ~2us back-edge). Only used by test.py to get a
    large, dispatch-noise-immune timing signal."""
    import concourse.bass as bass
    import concourse.bacc as bacc
    import concourse.mybir as mybir
    from concourse import tile
    from contextlib import nullcontext

    ng = NG
    fp32 = mybir.dt.float32
    bf16 = mybir.dt.bfloat16
    nc = bacc.Bacc(
        "TRN2", target_bir_lowering=False, debug=False, num_devices=N_CORES
    )

    x = nc.dram_tensor("x", [NPAIR, ng, T, PW], bf16, kind="ExternalInput")
    # amat[:, 0:128] = steady-state lhsT; amat[:, 128:256] = first-block lhsT.
    # The loop variant pads amat by loop_iters columns: the jax-side NEFF
    # cache fingerprints only the HLO (shapes), not the embedded BIR, so the
    # timing variant must differ in shape to avoid a false NEFF-cache hit.
    amat_cols = 256 + (loop_iters if loop_iters > 1 else 0)
    amat = nc.dram_tensor("amat", [128, amat_cols], bf16, kind="ExternalInput")
    y = nc.dram_tensor("y", [NPAIR, ng, T, PW], bf16, kind="ExternalOutput")
    x_ap, y_ap = x.ap(), y.ap()

    def pair3(ap_2d):
        """[P, PW] AP -> [P, 2, W] view (batch-in-pair axis split out)."""
        return ap_2d.rearrange("t (b f) -> t b f", b=2)

    with tile.TileContext(nc) as tc:
        with (
            tc.tile_pool(name="w", bufs=1) as w_pool,
            tc.tile_pool(name="rhs", bufs=2 * NPAIR) as rhs_pool,
            tc.tile_pool(name="out", bufs=2 * NPAIR) as out_pool,
            tc.tile_pool(name="psum", bufs=4, space="PSUM") as psum_pool,
        ):
            w = w_pool.tile([128, 256], bf16)
            nc.sync.dma_start(w[:, :], amat.ap()[:, 0:256])

            loop_cm = (
                tc.For_i(0, loop_iters, 1) if loop_iters > 1 else nullcontext()
            )

            def load_group(p, g):
                """rhs tile for (pair p, group g): contiguous DMA(s) into
                partitions 2..127; carries (partitions 0..1) are written by
                the previous block's carry copy. Group 0 is split into small
                head chunks so the first matmuls start after ~2us instead of
                waiting for the full 2.77 MB transfer."""
                rt = rhs_pool.tile([128, PW], bf16, name="rt")
                bounds = (0,) + tuple(load_split) + (GRP,) if g == 0 else (0, GRP)
                xg3 = x_ap[p, g, :, :].rearrange("t (b f) -> t b f", b=2)
                for lo, hi in zip(bounds[:-1], bounds[1:]):
                    cols = slice(lo * C, hi * C)
                    nc.sync.dma_start(
                        pair3(rt[2:128, :])[:, :, cols], xg3[:, :, cols]
                    )
                if g == 0:
                    # block 0 uses the A0 matrix (zero carry columns); its
                    # carry partitions just need to be finite
                    nc.gpsimd.memset(pair3(rt[0:2, :])[:, :, 0:C], 0.0)
                return rt

            with loop_cm:
                rhs_cur = [load_group(p, 0) for p in range(NPAIR)]

                for g in range(ng):
                    rhs_nxt = [
                        load_group(p, g + 1) if g + 1 < ng else None
                        for p in range(NPAIR)
                    ]
                    ots = [
                        out_pool.tile([128, PW], bf16, name="ot")
                        for p in range(NPAIR)
                    ]
                    for k in range(GRP):
                        for p in range(NPAIR):
                            rt = rhs_cur[p]
                            ps = psum_pool.tile([128, 2 * C], fp32)
                            first_block = g == 0 and k == 0
                            lhsT = w[:, 128:256] if first_block else w[:, 0:128]
                            # the pair's two blocks at position k -> the two
                            # halves of one 2-bank PSUM tile
                            nc.tensor.matmul(
                                ps[:, 0:C], lhsT, rt[:, k * C : (k + 1) * C],
                                start=True, stop=True,
                            )
                            nc.tensor.matmul(
                                ps[:, C : 2 * C], lhsT,
                                rt[:, W + k * C : W + (k + 1) * C],
                                start=True, stop=True,
                            )
                            if copy_scheme in (
                                "pair_act_dve", "split_carry", "carry_from_ot"
                            ):
                                out_eng = nc.scalar.copy
                                carry_eng = nc.vector.tensor_copy
                            elif copy_scheme == "chain_per_engine":
                                # chain p0 entirely on ACT, chain p1 on DVE:
                                # no head-of-line coupling between chains
                                out_eng = (
                                    nc.scalar.copy if p == 0
                                    else nc.vector.tensor_copy
                                )
                                carry_eng = out_eng
                            elif copy_scheme == "crossed":
                                out_eng = (
                                    nc.scalar.copy if p == 0
                                    else nc.vector.tensor_copy
                                )
                                carry_eng = (
                                    nc.vector.tensor_copy if p == 0
                                    else nc.scalar.copy
                                )
                            else:
                                raise ValueError(copy_scheme)
                            # ONE cast copy for both blocks (incl. carry rows,
                            # harmless)
                            out_eng(
                                pair3(ots[p][:, :])[:, :, k * C : (k + 1) * C],
                                pair3(ps[:, :]),
                            )
                            # carry copy/copies into the next block's rhs
                            if k + 1 < GRP:
                                ct, cols = rhs_cur[p], slice(
                                    (k + 1) * C, (k + 2) * C
                                )
                            elif rhs_nxt[p] is not None:
                                ct, cols = rhs_nxt[p], slice(0, C)
                            else:
                                ct = None
                            if ct is not None and copy_scheme == "carry_from_ot":
                                # carry from the already-cast bf16 out tile,
                                # bitcast to u32: DVE 2x_2P mode makes this
                                # ~3x cheaper than the fp32 PSUM read; bits
                                # identical to casting from PSUM directly
                                import concourse.mybir as mybir

                                src = pair3(ots[p][0:2, :])[
                                    :, :, k * C : (k + 1) * C
                                ].bitcast(mybir.dt.uint32)
                                carry_eng(
                                    pair3(ct[0:2, :])[:, :, cols].bitcast(
                                        mybir.dt.uint32
                                    ),
                                    src,
                                )
                            elif ct is not None and copy_scheme == "split_carry":
                                # per-batch carries: 4 independent short
                                # chains instead of 2 coupled ones
                                for b in range(2):
                                    carry_eng(
                                        ct[
                                            0:2,
                                            b * W + cols.start : b * W + cols.stop,
                                        ],
                                        ps[0:2, b * C : (b + 1) * C],
                                    )
                            elif ct is not None:
                                carry_eng(
                                    pair3(ct[0:2, :])[:, :, cols],
                                    pair3(ps[0:2, :]),
                                )
                        # store finished chunks early (SWDGE/Pool ring keeps
                        # store DMA time off the ACT/SP queues); splitting
                        # drains the last group incrementally
                        if k + 1 in store_split or k + 1 == GRP:
                            prev = max(
                                [0] + [s for s in store_split if s < k + 1]
                            )
                            cols = slice(prev * C, (k + 1) * C)
                            for p in range(NPAIR):
                                yg3 = y_ap[p, g, :, :].rearrange(
                                    "t (b f) -> t b f", b=2
                                )
                                nc.gpsimd.dma_start(
                                    yg3[:, :, cols],
                                    pair3(ots[p][2:128, :])[:, :, cols],
                                )
                    rhs_cur = rhs_nxt
    nc.compile()
    return nc


_CACHED = {}


def _get_nc():
    if "nc" not in _CACHED:
        _CACHED["nc"] = build_bass()
    return _CACHED["nc"]


def _core_in_maps(xdev, amat):
    return [
        {"x": xdev[i * NPAIR : (i + 1) * NPAIR], "amat": amat}
        for i in range(N_CORES)
    ]


def kernel(**inputs: np.ndarray) -> np.ndarray:
    from concourse.bass_utils import run_bass_kernel_spmd

    x = np.ascontiguousarray(inputs["x"], dtype=np.float32)
    assert x.shape == (B_FULL, L, C), x.shape

    xdev = _repack_x(x)
    amat = _amat_np()

    nc = _get_nc()
    res = run_bass_kernel_spmd(
        nc, _core_in_maps(xdev, amat), core_ids=list(range(N_CORES))
    )
    y_u16 = np.concatenate(
        [np.ascontiguousarray(r["y"]).view(np.uint16) for r in res.results],
        axis=0,
    )
    return _unpack_y(y_u16)
